# revision 55
# baseline (speedup 1.0000x reference)
"""Trainium2 Bass kernel for nn_Connector_77738908057780 (dense_mlp).

Computation (see reference):
  x   = image_features                      [B, N, H]    bf16
  f1  = mean(hidden[0:13],  axis=0)         [B, N, H]
  f2  = mean(hidden[13:26], axis=0)         [B, N, H]
  cat = concat([x, f1, f2], -1)             [B, N, 3H]
  h   = gelu(cat @ W1.T + b1)               W1 = nf4_dequant(codes1, scales1) [H, 3H]
  fg  = h @ W2.T + b2                       W2 = nf4_dequant(codes2, scales2) [H, H]
  out = w * LN(fg) + (1-w) * LN(x),         w = sigmoid(alpha)

Sharding: data-parallel over batch B=8 -> one batch element per NeuronCore.

Per-core schedule (v5, software-pipelined over 3 token supertiles of 256):
  - sync HWDGE queue carries only the dependency-free streaming loads in
    time order (x tile, 26 hidden layers per supertile, with the later
    GEMM1/GEMM2 weight chunks spliced in at the position they are first
    needed); the scalar HWDGE queue carries the first weight chunk +
    consts up front, then output stores.
  - 26-layer sums on DVE add-chains tracking the DMA stream; cat^T built
    by PE identity-transposes (PSUM) + ACT copies, so the DMA queues see
    no xbar transposes (dma_start_transpose drains its queue as deadlock
    protection, head-blocking the stream).
  - GEMM1 is k-OUTER with 9 m-chunk PSUM accumulators packed 2-per-bank
    (5 banks): the k=0..8 (x) matmuls run as soon as x lands, k=9..17 on
    s1, k=18..26 on s2 — so only ~10us of GEMM1 trails each supertile's
    stream instead of the whole GEMM.
  - GELU(+b1 per-partition bias) on ACT -> g^T feeds GEMM2 as stationary,
    producing fg token-major.
  - LN stats via DVE bn_stats/bn_aggr; rsqrt via DVE reciprocal + ACT
    sqrt. When the folded LN gains are uniform (ln*_g constant, gate
    scalar — true for this model's inputs) the normalize+gate combine
    runs as two ACT ops (per-partition scale/bias) + one DVE add; the
    general path (4 DVE scalar_tensor_tensor ops) is kept as a fallback
    program variant.

The last supertile overlaps the previous by 39 tokens (computed twice,
stored twice, identical) so every tile is a full 128-partition tile.

NF4 dequant of the (small, replicated) weights is host-side weight prep;
the bf16 weights are less DMA traffic than the int32 codes.
"""

import os
import sys

import numpy as np
import ml_dtypes

for _p in ("/opt/trn_rl_repo", "/root/.axon_site/_ro/trn_rl_repo"):
    if os.path.isdir(_p) and _p not in sys.path:
        sys.path.insert(0, _p)

import concourse.bass as bass
import concourse.mybir as mybir
import concourse.tile as tile
from concourse import bacc
from concourse import bass_utils
from concourse import masks

BF16 = mybir.dt.bfloat16
F32 = mybir.dt.float32
AF = mybir.ActivationFunctionType
ALU = mybir.AluOpType

NP_BF16 = ml_dtypes.bfloat16

P = 128
H = 1152
H3 = 3456
NT = 729          # tokens per core (N); B=8 cores
L = 26
KO1 = H3 // P     # 27 k-tiles for GEMM1
KO2 = H // P      # 9 k-tiles for GEMM2
MO = H // P       # 9 output-feature tiles
EPS = 1e-5
NCHUNK = 3        # fg free-dim chunks of 384
CH = H // NCHUNK  # 384

# Supertiles of 256 tokens; the last holds 217 (729 total). Its second
# subtile has 89 real rows; the remaining 39 rows of each input tile are
# gpsimd-memset to zero once so downstream 128-partition ops read
# initialized data (their outputs for those rows are never stored).
SUPS = [(0, 256), (256, 256), (512, 217)]
NSUB = 2

NF4_CODEBOOK = np.array([
    -1.0, -0.6961928009986877, -0.5250730514526367, -0.39491748809814453,
    -0.28444138169288635, -0.18477343022823334, -0.09105003625154495, 0.0,
    0.07958029955625534, 0.16093020141124725, 0.24611230194568634,
    0.33791524171829224, 0.4407098591327667, 0.5626170039176941,
    0.7229568362236023, 1.0], dtype=np.float32)

BLOCK = 64


def _dequant_nf4(codes, scales):
    """Match reference: codebook lookup * per-64-block absmax, cast bf16."""
    out_f, in_f = codes.shape
    w = NF4_CODEBOOK[codes].reshape(out_f, in_f // BLOCK, BLOCK)
    w = w * scales[:, :, None].astype(np.float32)
    return w.reshape(out_f, in_f)  # float32 (caller casts)


def _build_program(act=AF.Gelu, fast_ln=None):
    """fast_ln: None for the general LN-combine path, or (g1v, g2v) scalar
    gains (>0, with zero combined bias) for the ACT-based fast path."""
    nc = bacc.Bacc(
        "TRN2",
        target_bir_lowering=False,
        debug=False,
        num_devices=1,
    )
    x_d = nc.dram_tensor("x", (NT, H), BF16, kind="ExternalInput").ap()
    hid_d = nc.dram_tensor("hid", (L, NT, H), BF16, kind="ExternalInput").ap()
    w1t_d = nc.dram_tensor("w1t", (H3, H), BF16, kind="ExternalInput").ap()
    w2t_d = nc.dram_tensor("w2t", (H, H), BF16, kind="ExternalInput").ap()
    b1s_d = nc.dram_tensor("b1s", (P, MO), F32, kind="ExternalInput").ap()
    b2b_d = nc.dram_tensor("b2b", (1, H), BF16, kind="ExternalInput").ap()
    g1b_d = nc.dram_tensor("g1b", (P, H), BF16, kind="ExternalInput").ap()
    g2b_d = nc.dram_tensor("g2b", (P, H), BF16, kind="ExternalInput").ap()
    bcb_d = nc.dram_tensor("bcb", (P, H), BF16, kind="ExternalInput").ap()
    out_d = nc.dram_tensor("out", (NT, H), BF16, kind="ExternalOutput").ap()

    with tile.TileContext(nc) as tc:
        _program(nc, tc, x_d, hid_d, w1t_d, w2t_d, b1s_d, b2b_d,
                 g1b_d, g2b_d, bcb_d, out_d, act, fast_ln)

    nc.compile()
    return nc


def _program(nc, tc, x_d, hid_d, w1t_d, w2t_d, b1s_d, b2b_d, g1b_d, g2b_d,
             bcb_d, out_d, act=AF.Gelu, fast_ln=None):
    with (
        tc.tile_pool(name="consts", bufs=1) as cpool,
        tc.tile_pool(name="hl", bufs=9) as hpool,
        tc.tile_pool(name="acc", bufs=3) as apool,
        tc.tile_pool(name="cat", bufs=2) as catpool,
        tc.tile_pool(name="gt", bufs=2) as gpool,
        tc.tile_pool(name="xn", bufs=2) as xpool,
        tc.tile_pool(name="fg", bufs=4) as fgpool,
        tc.tile_pool(name="outp", bufs=2) as opool,
        tc.tile_pool(name="stats", bufs=3) as spool,
        tc.tile_pool(name="tmp", bufs=2) as tpool,
        tc.tile_pool(name="psA", bufs=5, space="PSUM") as ps1pool,
        tc.tile_pool(name="psB", bufs=2, space="PSUM") as ps2pool,
        tc.tile_pool(name="psT", bufs=1, space="PSUM") as tpspool,
    ):
        # Current supertile's views into the transpose-staging PSUM bank:
        # [0:384] bf16 staging (a transpose's start=True zeroes the whole
        # bank, so nothing live may share it), [384:386] the PE-warmer
        # scratch (written via accumulate, never read). Set by loads_a.
        cur_dmy = [None]
        cur_tps = [None]
        # ---- first GEMM1 weight chunk + small consts on the scalar HWDGE
        # queue up front; the remaining weight chunks are spliced into the
        # sync stream (see accum_half inline=).
        w1t_sb = cpool.tile([P, KO1, H], BF16, name="w1t")
        w1t_r = w1t_d.rearrange("(ko p) n -> p ko n", p=P)
        nc.scalar.dma_start(w1t_sb[:, 0:4, :], w1t_r[:, 0:4, :])
        nc.scalar.dma_start(w1t_sb[:, 4:9, :], w1t_r[:, 4:9, :])
        w2t_sb = cpool.tile([P, KO2, H], BF16, name="w2t")
        b1_sb = cpool.tile([P, MO], F32, name="b1s")
        nc.scalar.dma_start(b1_sb, b1s_d)
        b2b_sb = cpool.tile([1, H], BF16, name="b2b")
        nc.scalar.dma_start(b2b_sb, b2b_d[0:1, :])
        if fast_ln is None:
            g1b_sb = cpool.tile([P, H], BF16, name="g1b")
            nc.scalar.dma_start(g1b_sb, g1b_d)
            g2b_sb = cpool.tile([P, H], BF16, name="g2b")
            nc.scalar.dma_start(g2b_sb, g2b_d)
            bcb_sb = cpool.tile([P, H], BF16, name="bcb")
            nc.scalar.dma_start(bcb_sb, bcb_d)
        ident = cpool.tile([P, P], BF16, name="ident")
        masks.make_identity(nc, ident)
        ones_sb = cpool.tile([1, P], BF16, name="ones1")
        nc.vector.memset(ones_sb, 1.0)

        st = [dict() for _ in SUPS]

        def pe_transpose(catT, tt, ko0, src):
            """catT[:, tt, ko0+k, :] = src[:, k*P:(k+1)*P].T for k in 0..MO.

            PE identity-transpose into PSUM, ACT copies back to SBUF. Keeps
            the HWDGE queues free of xbar transposes (dma_start_transpose
            drains its queue as deadlock protection; measured, it stalls
            the streaming loads badly on either HWDGE queue).
            """
            for g in range(3):
                tps = cur_tps[0]
                for c in range(3):
                    k = g * 3 + c
                    nc.tensor.transpose(tps[:, c, :],
                                        src[:, k * P:(k + 1) * P], ident)
                nc.scalar.copy(catT[:, tt, ko0 + 3 * g:ko0 + 3 * (g + 1), :],
                               tps)

        def gemm1_ks(si, k0, k1):
            """GEMM1 matmuls for k-chunks [k0, k1) across all 9 m-chunks."""
            S = st[si]
            catT = S["catT"]
            ps1s = S["ps1s"]
            # Two m-chunks share each PSUM bank, so the hardware's
            # bank-granular start-zeroing cannot be used: the tiles are
            # memset once (loads_a) and every matmul pure-accumulates.
            for kk in range(k0, k1):
                for mm in range(MO):
                    dst = ps1s[mm // 2][:, mm % 2, :]
                    nc.tensor.matmul(
                        dst.rearrange("p (a b) -> p a b", a=NSUB),
                        lhsT=w1t_sb[:, kk, mm * P:(mm + 1) * P],
                        rhs=catT[:, :, kk, :],
                        start=False,
                        stop=(kk == KO1 - 1),
                        skip_group_check=True,
                    )

        def load_token_tile(dst, src2d, t0, ntok):
            """Load [ntok, H] DRAM rows into dst [P, 2, H] token-major.
            For partial supertiles the caller pre-memsets rows ntok-P..P of
            subtile 1 (gpsimd), so every downstream 128-partition op reads
            initialized data."""
            if ntok == 2 * P:
                nc.sync.dma_start(
                    dst, src2d[t0:t0 + 2 * P, :].rearrange(
                        "(s p) f -> p s f", p=P))
            else:
                nc.sync.dma_start(dst[:, 0, :], src2d[t0:t0 + P, :])
                nc.sync.dma_start(dst[0:ntok - P, 1, :],
                                  src2d[t0 + P:t0 + ntok, :])

        def accum_half(si, half, insert_s1T=False, inline=None):
            """Load 13 layers, DVE-chain them into an acc tile.

            ``inline`` maps layer-offset -> callable emitted right after that
            layer's load (used to splice weight-chunk DMAs into the sync
            stream at the position they are first needed).
            """
            t0, ntok = SUPS[si]
            l0 = 13 * half
            acc = apool.tile([P, NSUB, H], BF16, name=f"s{si}_{half}",
                             tag="acc")
            def layer_tile(i):
                lt = hpool.tile([P, NSUB, H], BF16, name=f"hl{si}_{l0+i}",
                                tag="hl")
                if ntok < 2 * P:
                    # partition slices must be 32-aligned; zero 64..128 and
                    # let the DMA overwrite the real rows 64..ntok-P
                    nc.gpsimd.memset(lt[64:P, 1, :], 0.0)
                load_token_tile(lt, hid_d[l0 + i], t0, ntok)
                if inline and i in inline:
                    inline[i]()
                return lt

            lts = [layer_tile(0), layer_tile(1)]
            nc.vector.tensor_add(acc, lts[0], lts[1])
            for i in range(2, 13):
                lt = layer_tile(i)
                nc.vector.tensor_add(acc, acc, lt)
                if insert_s1T and i == 3:
                    # s1 is long done by now; transpose it mid-stream and
                    # run GEMM1's k=9..17 before s2 lands.
                    S = st[si]
                    for tt in range(NSUB):
                        pe_transpose(S["catT"], tt, MO, S["s1"][:, tt, :])
                    gemm1_ks(si, MO, 2 * MO)
            return acc

        def loads_a(si):
            t0, ntok = SUPS[si]
            S = st[si]
            catT = catpool.tile([P, NSUB, KO1, P], BF16, name=f"cat{si}",
                                tag="catT")
            S["catT"] = catT
            S["ps1s"] = [ps1pool.tile([P, 2, NSUB * P], F32, tag="ps1",
                                      name=f"ps1_{si}_{j}")
                         for j in range(5)]
            tpsf = tpspool.tile([P, 400], BF16, tag="tps",
                                name=f"tps{si}")
            cur_tps[0] = tpsf[:, 0:3 * P].rearrange("p (a b) -> p a b", a=3)
            cur_dmy[0] = tpsf[0:1, 384:386].bitcast(F32)
            for t in S["ps1s"]:
                nc.vector.memset(t, 0.0)
            x_nat = xpool.tile([P, NSUB, H], BF16, name=f"x{si}", tag="xnat")
            S["x"] = x_nat
            if ntok < 2 * P:
                nc.gpsimd.memset(x_nat[64:P, 1, :], 0.0)
            load_token_tile(x_nat, x_d, t0, ntok)
            for tt in range(NSUB):
                pe_transpose(catT, tt, 0, x_nat[:, tt, :])
            gemm1_ks(si, 0, MO)
            agg = spool.tile([P, NSUB, 4], F32, name=f"agg{si}", tag="agg")
            S["agg"] = agg
            for tt in range(NSUB):
                bnx = spool.tile([P, 3, 6], F32, name=f"bnx{si}_{tt}",
                                 tag="bnx")
                for c in range(NCHUNK):
                    nc.vector.bn_stats(bnx[:, c, :],
                                       x_nat[:, tt, c * CH:(c + 1) * CH])
                nc.vector.bn_aggr(agg[:, tt, 0:2], bnx)
            inline = None
            if si == 0:
                inline = {6: lambda: nc.sync.dma_start(
                    w1t_sb[:, 9:18, :], w1t_r[:, 9:18, :])}
            S["s1"] = accum_half(si, 0, inline=inline)

        def loads_b(si):
            S = st[si]
            inline = None
            if si == 0:
                inline = {
                    2: lambda: nc.sync.dma_start(
                        w1t_sb[:, 18:27, :], w1t_r[:, 18:27, :]),
                    7: lambda: nc.sync.dma_start(
                        w2t_sb, w2t_d.rearrange("(ko p) n -> p ko n", p=P)),
                }
            S["s2"] = accum_half(si, 1, insert_s1T=True, inline=inline)
            for tt in range(NSUB):
                pe_transpose(S["catT"], tt, 2 * MO, S["s2"][:, tt, :])

        def tail_pe(si):
            S = st[si]
            gemm1_ks(si, 2 * MO, 3 * MO)
            gT = gpool.tile([P, MO, NSUB * P], BF16, name=f"gT{si}", tag="gT")
            S["gT"] = gT
            for mm in range(MO):
                nc.scalar.activation(gT[:, mm, :],
                                     S["ps1s"][mm // 2][:, mm % 2, :],
                                     act, bias=b1_sb[:, mm:mm + 1])
            fgs = []
            for tt in range(NSUB):
                fg = fgpool.tile([P, H], BF16, name=f"fg{si}_{tt}", tag="fg")
                fgs.append(fg)
                for nn in range(NCHUNK):
                    ps2 = ps2pool.tile([P, CH], F32, tag="ps2")
                    for kk in range(KO2):
                        nc.tensor.matmul(
                            ps2,
                            lhsT=gT[:, kk, tt * P:(tt + 1) * P],
                            rhs=w2t_sb[:, kk, nn * CH:(nn + 1) * CH],
                            start=(kk == 0),
                            stop=False,
                        )
                    # b2 bias as a k=1 ones-row matmul; evacuation on ACT
                    # (keeps the bias-add off the saturated DVE)
                    nc.tensor.matmul(
                        ps2,
                        lhsT=ones_sb,
                        rhs=b2b_sb[:, nn * CH:(nn + 1) * CH],
                        start=False,
                        stop=True,
                    )
                    nc.scalar.copy(fg[:, nn * CH:(nn + 1) * CH], ps2)
            S["fgs"] = fgs

        def tail_dve(si):
            t0, ntok = SUPS[si]
            S = st[si]
            agg = S["agg"]
            rpack = spool.tile([P, 2 * NSUB], F32, name=f"rp{si}", tag="rpack")
            for tt in range(NSUB):
                fg = S["fgs"][tt]
                bnf = spool.tile([P, 3, 6], F32, name=f"bnf{si}_{tt}",
                                 tag="bnf")
                for c in range(NCHUNK):
                    nc.vector.bn_stats(bnf[:, c, :],
                                       fg[:, c * CH:(c + 1) * CH])
                nc.vector.bn_aggr(agg[:, tt, 2:4], bnf)
                if fast_ln is None:
                    nc.vector.tensor_scalar_add(rpack[:, 2 * tt:2 * tt + 1],
                                                agg[:, tt, 1:2], EPS)
                    nc.vector.tensor_scalar_add(
                        rpack[:, 2 * tt + 1:2 * tt + 2],
                        agg[:, tt, 3:4], EPS)
                else:
                    g1v, g2v = fast_ln
                    # rsqrt((var+eps)/g^2) = g * rsqrt(var+eps)
                    nc.vector.tensor_scalar(
                        rpack[:, 2 * tt:2 * tt + 1], agg[:, tt, 1:2],
                        EPS, float(1.0 / (g1v * g1v)), ALU.add, ALU.mult)
                    nc.vector.tensor_scalar(
                        rpack[:, 2 * tt + 1:2 * tt + 2], agg[:, tt, 3:4],
                        EPS, float(1.0 / (g2v * g2v)), ALU.add, ALU.mult)
            ig = spool.tile([P, 2 * NSUB], F32, name=f"ig{si}", tag="ig")
            nc.vector.reciprocal(ig, rpack)
            nc.scalar.activation(ig, ig, AF.Sqrt)
            for tt in range(NSUB):
                fg = S["fgs"][tt]
                out_t = opool.tile([P, H], BF16, name=f"o{si}_{tt}",
                                   tag="outt")
                if fast_ln is not None:
                    # u1 = (x - mu1) * igA computed as x*igA + (-mu1*igA)
                    # on ACT (per-partition scale/bias); same for fg; then
                    # one DVE add. igA/igB already carry the gains.
                    bias2 = spool.tile([P, 2], F32, name=f"bs{si}_{tt}",
                                       tag="bias2")
                    nc.vector.scalar_tensor_tensor(
                        bias2[:, 0:1], agg[:, tt, 0:1], -1.0,
                        ig[:, 2 * tt:2 * tt + 1], ALU.mult, ALU.mult)
                    nc.vector.scalar_tensor_tensor(
                        bias2[:, 1:2], agg[:, tt, 2:3], -1.0,
                        ig[:, 2 * tt + 1:2 * tt + 2], ALU.mult, ALU.mult)
                    u1 = tpool.tile([P, H], BF16, tag="tmp1")
                    u2 = tpool.tile([P, H], BF16, tag="tmp2")
                    nc.scalar.activation(u1, S["x"][:, tt, :], AF.Identity,
                                         bias=bias2[:, 0:1],
                                         scale=ig[:, 2 * tt:2 * tt + 1])
                    nc.scalar.activation(u2, fg, AF.Identity,
                                         bias=bias2[:, 1:2],
                                         scale=ig[:, 2 * tt + 1:2 * tt + 2])
                    nc.vector.tensor_tensor(out_t, u1, u2, ALU.add)
                else:
                    tmp1 = tpool.tile([P, H], BF16, tag="tmp1")
                    tmp2 = tpool.tile([P, H], BF16, tag="tmp2")
                    # tmp2 = (fg - mu2) * G2;  G2 = w*ln2_g broadcast
                    nc.vector.scalar_tensor_tensor(
                        tmp2, fg, agg[:, tt, 2:3], g2b_sb,
                        ALU.subtract, ALU.mult)
                    # tmp1 = (x - mu1) * G1;  G1 = (1-w)*ln1_g
                    nc.vector.scalar_tensor_tensor(
                        tmp1, S["x"][:, tt, :], agg[:, tt, 0:1], g1b_sb,
                        ALU.subtract, ALU.mult)
                    # tmp1 = tmp1 * ig1 + Bc;  Bc = w*ln2_b + (1-w)*ln1_b
                    nc.vector.scalar_tensor_tensor(
                        tmp1, tmp1, ig[:, 2 * tt:2 * tt + 1], bcb_sb,
                        ALU.mult, ALU.add)
                    nc.vector.scalar_tensor_tensor(
                        out_t, tmp2, ig[:, 2 * tt + 1:2 * tt + 2], tmp1,
                        ALU.mult, ALU.add)
                rows = P if (tt == 0 or ntok == 2 * P) else ntok - P
                nc.sync.dma_start(
                    out_d[t0 + tt * P:t0 + tt * P + rows, :],
                    out_t[0:rows, :])

        for si in range(len(SUPS)):
            loads_a(si)
            loads_b(si)
            if si > 0:
                tail_dve(si - 1)
            tail_pe(si)
        tail_dve(len(SUPS) - 1)


_NC_CACHE = {}


def _get_nc(fast_ln=None):
    key = ("fast", fast_ln) if fast_ln is None else (
        "fast", (round(float(fast_ln[0]), 6), round(float(fast_ln[1]), 6)))
    if key not in _NC_CACHE:
        _NC_CACHE[key] = _build_program(fast_ln=fast_ln)
    return _NC_CACHE[key]


def _host_prep(codes1, scales1, b1, codes2, scales2, b2,
               ln1_g, ln1_b, ln2_g, ln2_b, alpha):
    # W1 with 1/13 folded into the f1/f2 column blocks (mean -> sum)
    w1 = _dequant_nf4(codes1, scales1)
    # match reference rounding: dequant result is cast to bf16 first
    w1 = w1.astype(NP_BF16).astype(np.float32)
    w1[:, H:] *= np.float32(1.0 / 13.0)
    w1t = np.ascontiguousarray(w1.T).astype(NP_BF16)

    w2 = _dequant_nf4(codes2, scales2).astype(NP_BF16)
    w2t = np.ascontiguousarray(w2.astype(np.float32).T).astype(NP_BF16)

    b1s = np.ascontiguousarray(
        b1.astype(np.float32).reshape(MO, P).T)  # [P, MO]

    b2b = np.ascontiguousarray(b2.astype(NP_BF16).reshape(1, H))

    a32 = alpha.astype(np.float32)
    w_gate = (1.0 / (1.0 + np.exp(-a32[0]))).astype(NP_BF16)
    one_minus = (NP_BF16(1.0) - w_gate)
    g1 = (one_minus.astype(np.float32) * ln1_g.astype(np.float32))
    g2 = (w_gate.astype(np.float32) * ln2_g.astype(np.float32))
    bc = (w_gate.astype(np.float32) * ln2_b.astype(np.float32)
          + one_minus.astype(np.float32) * ln1_b.astype(np.float32))
    g1b = np.ascontiguousarray(np.broadcast_to(g1.astype(NP_BF16), (P, H)))
    g2b = np.ascontiguousarray(np.broadcast_to(g2.astype(NP_BF16), (P, H)))
    bcb = np.ascontiguousarray(np.broadcast_to(bc.astype(NP_BF16), (P, H)))

    # fast-LN eligibility: uniform positive folded gains, zero folded bias
    fast_ln = None
    g1f = g1.astype(NP_BF16).astype(np.float32)
    g2f = g2.astype(NP_BF16).astype(np.float32)
    if (np.all(g1f == g1f[0]) and np.all(g2f == g2f[0])
            and g1f[0] > 0 and g2f[0] > 0
            and np.all(bc.astype(NP_BF16) == 0)):
        fast_ln = (float(g1f[0]), float(g2f[0]))
    return w1t, w2t, b1s, b2b, g1b, g2b, bcb, fast_ln


def make_in_maps(image_features, hidden, codes1, scales1, b1, codes2, scales2,
                 b2, ln1_g, ln1_b, ln2_g, ln2_b, alpha):
    w1t, w2t, b1s, b2b, g1b, g2b, bcb, fast_ln = _host_prep(
        codes1, scales1, b1, codes2, scales2, b2,
        ln1_g, ln1_b, ln2_g, ln2_b, alpha)
    B = image_features.shape[0]
    in_maps = []
    for c in range(B):
        in_maps.append({
            "x": np.ascontiguousarray(image_features[c]).astype(NP_BF16, copy=False),
            "hid": np.ascontiguousarray(hidden[:, c]).astype(NP_BF16, copy=False),
            "w1t": w1t, "w2t": w2t, "b1s": b1s, "b2b": b2b,
            "g1b": g1b, "g2b": g2b, "bcb": bcb,
        })
    return in_maps, fast_ln


def kernel(image_features, hidden, codes1, scales1, b1, codes2, scales2, b2,
           ln1_g, ln1_b, ln2_g, ln2_b, alpha, _trace=False):
    B, N, Hin = image_features.shape
    assert (B, N, Hin) == (8, NT, H), (B, N, Hin)
    in_maps, fast_ln = make_in_maps(
        image_features, hidden, codes1, scales1, b1, codes2, scales2, b2,
        ln1_g, ln1_b, ln2_g, ln2_b, alpha)
    nc = _get_nc(fast_ln)
    res = bass_utils.run_bass_kernel_spmd(
        nc, in_maps, core_ids=list(range(8)), trace=_trace)
    out = np.stack([res.results[c]["out"] for c in range(8)])
    if _trace:
        kernel._last_results = res
    return out.astype(image_features.dtype, copy=False)


# revision 56
# speedup vs baseline: 1.7497x; 1.7497x over previous
"""Trainium2 Bass kernel for nn_Connector_77738908057780 (dense_mlp).

Computation (see reference):
  x   = image_features                      [B, N, H]    bf16
  f1  = mean(hidden[0:13],  axis=0)         [B, N, H]
  f2  = mean(hidden[13:26], axis=0)         [B, N, H]
  cat = concat([x, f1, f2], -1)             [B, N, 3H]
  h   = gelu(cat @ W1.T + b1)               W1 = nf4_dequant(codes1, scales1) [H, 3H]
  fg  = h @ W2.T + b2                       W2 = nf4_dequant(codes2, scales2) [H, H]
  out = w * LN(fg) + (1-w) * LN(x),         w = sigmoid(alpha)

Sharding: data-parallel over batch B=8 -> one batch element per NeuronCore.

Per-core schedule (v5, software-pipelined over 3 token supertiles of 256):
  - sync HWDGE queue carries only the dependency-free streaming loads in
    time order (x tile, 26 hidden layers per supertile, with the later
    GEMM1/GEMM2 weight chunks spliced in at the position they are first
    needed); the scalar HWDGE queue carries the first weight chunk +
    consts up front, then output stores.
  - 26-layer sums on DVE add-chains tracking the DMA stream; cat^T built
    by PE identity-transposes (PSUM) + ACT copies, so the DMA queues see
    no xbar transposes (dma_start_transpose drains its queue as deadlock
    protection, head-blocking the stream).
  - GEMM1 is k-OUTER with 9 m-chunk PSUM accumulators packed 2-per-bank
    (5 banks): the k=0..8 (x) matmuls run as soon as x lands, k=9..17 on
    s1, k=18..26 on s2 — so only ~10us of GEMM1 trails each supertile's
    stream instead of the whole GEMM.
  - GELU(+b1 per-partition bias) on ACT -> g^T feeds GEMM2 as stationary,
    producing fg token-major.
  - LN stats via DVE bn_stats/bn_aggr; rsqrt via DVE reciprocal + ACT
    sqrt. When the folded LN gains are uniform (ln*_g constant, gate
    scalar — true for this model's inputs) the normalize+gate combine
    runs as two ACT ops (per-partition scale/bias) + one DVE add; the
    general path (4 DVE scalar_tensor_tensor ops) is kept as a fallback
    program variant.

The last supertile overlaps the previous by 39 tokens (computed twice,
stored twice, identical) so every tile is a full 128-partition tile.

NF4 dequant of the (small, replicated) weights is host-side weight prep;
the bf16 weights are less DMA traffic than the int32 codes.
"""

import os
import sys

import numpy as np
import ml_dtypes

for _p in ("/opt/trn_rl_repo", "/root/.axon_site/_ro/trn_rl_repo"):
    if os.path.isdir(_p) and _p not in sys.path:
        sys.path.insert(0, _p)

import concourse.bass as bass
import concourse.mybir as mybir
import concourse.tile as tile
from concourse import bacc
from concourse import bass_utils
from concourse import masks

BF16 = mybir.dt.bfloat16
F32 = mybir.dt.float32
AF = mybir.ActivationFunctionType
ALU = mybir.AluOpType

NP_BF16 = ml_dtypes.bfloat16

P = 128
H = 1152
H3 = 3456
NT = 729          # tokens per core (N); B=8 cores
L = 26
KO1 = H3 // P     # 27 k-tiles for GEMM1
KO2 = H // P      # 9 k-tiles for GEMM2
MO = H // P       # 9 output-feature tiles
EPS = 1e-5
NCHUNK = 3        # fg free-dim chunks of 384
CH = H // NCHUNK  # 384

# Supertiles of exactly 256 tokens; the last overlaps the previous by 39
# tokens (473..511 computed twice, identical values stored twice) so that
# every DMA/compute tile is a full 128-partition tile (729 is not a
# multiple of 128; partial tiles would leave stale SBUF rows flowing into
# the transposes and stats ops).
SUPS = [(0, 256), (256, 256), (473, 256)]
NSUB = 2

NF4_CODEBOOK = np.array([
    -1.0, -0.6961928009986877, -0.5250730514526367, -0.39491748809814453,
    -0.28444138169288635, -0.18477343022823334, -0.09105003625154495, 0.0,
    0.07958029955625534, 0.16093020141124725, 0.24611230194568634,
    0.33791524171829224, 0.4407098591327667, 0.5626170039176941,
    0.7229568362236023, 1.0], dtype=np.float32)

BLOCK = 64


def _dequant_nf4(codes, scales):
    """Match reference: codebook lookup * per-64-block absmax, cast bf16."""
    out_f, in_f = codes.shape
    w = NF4_CODEBOOK[codes].reshape(out_f, in_f // BLOCK, BLOCK)
    w = w * scales[:, :, None].astype(np.float32)
    return w.reshape(out_f, in_f)  # float32 (caller casts)


def _build_program(act=AF.Gelu, fast_ln=None):
    """fast_ln: None for the general LN-combine path, or (g1v, g2v) scalar
    gains (>0, with zero combined bias) for the ACT-based fast path."""
    nc = bacc.Bacc(
        "TRN2",
        target_bir_lowering=False,
        debug=False,
        num_devices=1,
    )
    x_d = nc.dram_tensor("x", (NT, H), BF16, kind="ExternalInput").ap()
    hid_d = nc.dram_tensor("hid", (L, NT, H), BF16, kind="ExternalInput").ap()
    w1t_d = nc.dram_tensor("w1t", (H3, H), BF16, kind="ExternalInput").ap()
    w2t_d = nc.dram_tensor("w2t", (H, H), BF16, kind="ExternalInput").ap()
    b1s_d = nc.dram_tensor("b1s", (P, MO), F32, kind="ExternalInput").ap()
    b2b_d = nc.dram_tensor("b2b", (1, H), BF16, kind="ExternalInput").ap()
    g1b_d = nc.dram_tensor("g1b", (P, H), BF16, kind="ExternalInput").ap()
    g2b_d = nc.dram_tensor("g2b", (P, H), BF16, kind="ExternalInput").ap()
    bcb_d = nc.dram_tensor("bcb", (P, H), BF16, kind="ExternalInput").ap()
    out_d = nc.dram_tensor("out", (NT, H), BF16, kind="ExternalOutput").ap()

    with tile.TileContext(nc) as tc:
        _program(nc, tc, x_d, hid_d, w1t_d, w2t_d, b1s_d, b2b_d,
                 g1b_d, g2b_d, bcb_d, out_d, act, fast_ln)

    nc.compile()
    return nc


def _program(nc, tc, x_d, hid_d, w1t_d, w2t_d, b1s_d, b2b_d, g1b_d, g2b_d,
             bcb_d, out_d, act=AF.Gelu, fast_ln=None):
    with (
        tc.tile_pool(name="consts", bufs=1) as cpool,
        tc.tile_pool(name="hl", bufs=9) as hpool,
        tc.tile_pool(name="acc", bufs=3) as apool,
        tc.tile_pool(name="cat", bufs=2) as catpool,
        tc.tile_pool(name="gt", bufs=2) as gpool,
        tc.tile_pool(name="xn", bufs=2) as xpool,
        tc.tile_pool(name="fg", bufs=4) as fgpool,
        tc.tile_pool(name="outp", bufs=2) as opool,
        tc.tile_pool(name="stats", bufs=3) as spool,
        tc.tile_pool(name="tmp", bufs=2) as tpool,
        tc.tile_pool(name="psA", bufs=5, space="PSUM") as ps1pool,
        tc.tile_pool(name="psB", bufs=2, space="PSUM") as ps2pool,
        tc.tile_pool(name="psT", bufs=1, space="PSUM") as tpspool,
    ):
        # Current supertile's views into the transpose-staging PSUM bank:
        # [0:384] bf16 staging (a transpose's start=True zeroes the whole
        # bank, so nothing live may share it), [384:386] the PE-warmer
        # scratch (written via accumulate, never read). Set by loads_a.
        cur_dmy = [None]
        cur_tps = [None]
        # ---- first GEMM1 weight chunk + small consts on the scalar HWDGE
        # queue up front; the remaining weight chunks are spliced into the
        # sync stream (see accum_half inline=).
        w1t_sb = cpool.tile([P, KO1, H], BF16, name="w1t")
        w1t_r = w1t_d.rearrange("(ko p) n -> p ko n", p=P)
        nc.scalar.dma_start(w1t_sb[:, 0:4, :], w1t_r[:, 0:4, :])
        nc.scalar.dma_start(w1t_sb[:, 4:9, :], w1t_r[:, 4:9, :])
        w2t_sb = cpool.tile([P, KO2, H], BF16, name="w2t")
        b1_sb = cpool.tile([P, MO], F32, name="b1s")
        nc.scalar.dma_start(b1_sb, b1s_d)
        b2b_sb = cpool.tile([1, H], BF16, name="b2b")
        nc.scalar.dma_start(b2b_sb, b2b_d[0:1, :])
        if fast_ln is None:
            g1b_sb = cpool.tile([P, H], BF16, name="g1b")
            nc.scalar.dma_start(g1b_sb, g1b_d)
            g2b_sb = cpool.tile([P, H], BF16, name="g2b")
            nc.scalar.dma_start(g2b_sb, g2b_d)
            bcb_sb = cpool.tile([P, H], BF16, name="bcb")
            nc.scalar.dma_start(bcb_sb, bcb_d)
        ident = cpool.tile([P, P], BF16, name="ident")
        masks.make_identity(nc, ident)
        ones_sb = cpool.tile([1, P], BF16, name="ones1")
        nc.vector.memset(ones_sb, 1.0)

        st = [dict() for _ in SUPS]

        def pe_transpose(catT, tt, ko0, src):
            """catT[:, tt, ko0+k, :] = src[:, k*P:(k+1)*P].T for k in 0..MO.

            PE identity-transpose into PSUM, ACT copies back to SBUF. Keeps
            the HWDGE queues free of xbar transposes (dma_start_transpose
            drains its queue as deadlock protection; measured, it stalls
            the streaming loads badly on either HWDGE queue).
            """
            for g in range(3):
                tps = cur_tps[0]
                for c in range(3):
                    k = g * 3 + c
                    nc.tensor.transpose(tps[:, c, :],
                                        src[:, k * P:(k + 1) * P], ident)
                nc.scalar.copy(catT[:, tt, ko0 + 3 * g:ko0 + 3 * (g + 1), :],
                               tps)

        def gemm1_ks(si, k0, k1):
            """GEMM1 matmuls for k-chunks [k0, k1) across all 9 m-chunks."""
            S = st[si]
            catT = S["catT"]
            ps1s = S["ps1s"]
            # Two m-chunks share each PSUM bank, so the hardware's
            # bank-granular start-zeroing cannot be used: the tiles are
            # memset once (loads_a) and every matmul pure-accumulates.
            for kk in range(k0, k1):
                for mm in range(MO):
                    dst = ps1s[mm // 2][:, mm % 2, :]
                    nc.tensor.matmul(
                        dst.rearrange("p (a b) -> p a b", a=NSUB),
                        lhsT=w1t_sb[:, kk, mm * P:(mm + 1) * P],
                        rhs=catT[:, :, kk, :],
                        start=False,
                        stop=(kk == KO1 - 1),
                        skip_group_check=True,
                    )

        def load_token_tile(dst, src2d, t0, ntok):
            """Load [ntok, H] DRAM rows into dst [P, 2, H] token-major.
            For partial supertiles the caller pre-memsets rows ntok-P..P of
            subtile 1 (gpsimd), so every downstream 128-partition op reads
            initialized data."""
            if ntok == 2 * P:
                nc.sync.dma_start(
                    dst, src2d[t0:t0 + 2 * P, :].rearrange(
                        "(s p) f -> p s f", p=P))
            else:
                nc.sync.dma_start(dst[:, 0, :], src2d[t0:t0 + P, :])
                nc.sync.dma_start(dst[0:ntok - P, 1, :],
                                  src2d[t0 + P:t0 + ntok, :])

        def accum_half(si, half, insert_s1T=False, inline=None):
            """Load 13 layers, DVE-chain them into an acc tile.

            ``inline`` maps layer-offset -> callable emitted right after that
            layer's load (used to splice weight-chunk DMAs into the sync
            stream at the position they are first needed).
            """
            t0, ntok = SUPS[si]
            l0 = 13 * half
            acc = apool.tile([P, NSUB, H], BF16, name=f"s{si}_{half}",
                             tag="acc")
            def layer_tile(i):
                lt = hpool.tile([P, NSUB, H], BF16, name=f"hl{si}_{l0+i}",
                                tag="hl")
                load_token_tile(lt, hid_d[l0 + i], t0, ntok)
                if inline and i in inline:
                    inline[i]()
                return lt

            lts = [layer_tile(0), layer_tile(1)]
            nc.vector.tensor_add(acc, lts[0], lts[1])
            for i in range(2, 13):
                lt = layer_tile(i)
                nc.vector.tensor_add(acc, acc, lt)
                if insert_s1T and i == 3:
                    # s1 is long done by now; transpose it mid-stream and
                    # run GEMM1's k=9..17 before s2 lands.
                    S = st[si]
                    for tt in range(NSUB):
                        pe_transpose(S["catT"], tt, MO, S["s1"][:, tt, :])
                    gemm1_ks(si, MO, 2 * MO)
            return acc

        def loads_a(si):
            t0, ntok = SUPS[si]
            S = st[si]
            catT = catpool.tile([P, NSUB, KO1, P], BF16, name=f"cat{si}",
                                tag="catT")
            S["catT"] = catT
            S["ps1s"] = [ps1pool.tile([P, 2, NSUB * P], F32, tag="ps1",
                                      name=f"ps1_{si}_{j}")
                         for j in range(5)]
            tpsf = tpspool.tile([P, 400], BF16, tag="tps",
                                name=f"tps{si}")
            cur_tps[0] = tpsf[:, 0:3 * P].rearrange("p (a b) -> p a b", a=3)
            cur_dmy[0] = tpsf[0:1, 384:386].bitcast(F32)
            for t in S["ps1s"]:
                nc.vector.memset(t, 0.0)
            x_nat = xpool.tile([P, NSUB, H], BF16, name=f"x{si}", tag="xnat")
            S["x"] = x_nat
            load_token_tile(x_nat, x_d, t0, ntok)
            for tt in range(NSUB):
                pe_transpose(catT, tt, 0, x_nat[:, tt, :])
            gemm1_ks(si, 0, MO)
            agg = spool.tile([P, NSUB, 4], F32, name=f"agg{si}", tag="agg")
            S["agg"] = agg
            for tt in range(NSUB):
                bnx = spool.tile([P, 3, 6], F32, name=f"bnx{si}_{tt}",
                                 tag="bnx")
                for c in range(NCHUNK):
                    nc.vector.bn_stats(bnx[:, c, :],
                                       x_nat[:, tt, c * CH:(c + 1) * CH])
                nc.vector.bn_aggr(agg[:, tt, 0:2], bnx)
            inline = None
            if si == 0:
                inline = {6: lambda: nc.sync.dma_start(
                    w1t_sb[:, 9:18, :], w1t_r[:, 9:18, :])}
            S["s1"] = accum_half(si, 0, inline=inline)

        def loads_b(si):
            S = st[si]
            inline = None
            if si == 0:
                inline = {
                    2: lambda: nc.sync.dma_start(
                        w1t_sb[:, 18:27, :], w1t_r[:, 18:27, :]),
                    7: lambda: nc.sync.dma_start(
                        w2t_sb, w2t_d.rearrange("(ko p) n -> p ko n", p=P)),
                }
            S["s2"] = accum_half(si, 1, insert_s1T=True, inline=inline)
            for tt in range(NSUB):
                pe_transpose(S["catT"], tt, 2 * MO, S["s2"][:, tt, :])

        def tail_pe(si):
            S = st[si]
            gemm1_ks(si, 2 * MO, 3 * MO)
            gT = gpool.tile([P, MO, NSUB * P], BF16, name=f"gT{si}", tag="gT")
            S["gT"] = gT
            for mm in range(MO):
                nc.scalar.activation(gT[:, mm, :],
                                     S["ps1s"][mm // 2][:, mm % 2, :],
                                     act, bias=b1_sb[:, mm:mm + 1])
            fgs = []
            for tt in range(NSUB):
                fg = fgpool.tile([P, H], BF16, name=f"fg{si}_{tt}", tag="fg")
                fgs.append(fg)
                for nn in range(NCHUNK):
                    ps2 = ps2pool.tile([P, CH], F32, tag="ps2")
                    for kk in range(KO2):
                        nc.tensor.matmul(
                            ps2,
                            lhsT=gT[:, kk, tt * P:(tt + 1) * P],
                            rhs=w2t_sb[:, kk, nn * CH:(nn + 1) * CH],
                            start=(kk == 0),
                            stop=False,
                        )
                    # b2 bias as a k=1 ones-row matmul; evacuation on ACT
                    # (keeps the bias-add off the saturated DVE)
                    nc.tensor.matmul(
                        ps2,
                        lhsT=ones_sb,
                        rhs=b2b_sb[:, nn * CH:(nn + 1) * CH],
                        start=False,
                        stop=True,
                    )
                    nc.scalar.copy(fg[:, nn * CH:(nn + 1) * CH], ps2)
            S["fgs"] = fgs

        def tail_dve(si):
            t0, ntok = SUPS[si]
            S = st[si]
            agg = S["agg"]
            rpack = spool.tile([P, 2 * NSUB], F32, name=f"rp{si}", tag="rpack")
            for tt in range(NSUB):
                fg = S["fgs"][tt]
                bnf = spool.tile([P, 3, 6], F32, name=f"bnf{si}_{tt}",
                                 tag="bnf")
                for c in range(NCHUNK):
                    nc.vector.bn_stats(bnf[:, c, :],
                                       fg[:, c * CH:(c + 1) * CH])
                nc.vector.bn_aggr(agg[:, tt, 2:4], bnf)
                if fast_ln is None:
                    nc.vector.tensor_scalar_add(rpack[:, 2 * tt:2 * tt + 1],
                                                agg[:, tt, 1:2], EPS)
                    nc.vector.tensor_scalar_add(
                        rpack[:, 2 * tt + 1:2 * tt + 2],
                        agg[:, tt, 3:4], EPS)
                else:
                    g1v, g2v = fast_ln
                    # rsqrt((var+eps)/g^2) = g * rsqrt(var+eps)
                    nc.vector.tensor_scalar(
                        rpack[:, 2 * tt:2 * tt + 1], agg[:, tt, 1:2],
                        EPS, float(1.0 / (g1v * g1v)), ALU.add, ALU.mult)
                    nc.vector.tensor_scalar(
                        rpack[:, 2 * tt + 1:2 * tt + 2], agg[:, tt, 3:4],
                        EPS, float(1.0 / (g2v * g2v)), ALU.add, ALU.mult)
            ig = spool.tile([P, 2 * NSUB], F32, name=f"ig{si}", tag="ig")
            nc.vector.reciprocal(ig, rpack)
            nc.scalar.activation(ig, ig, AF.Sqrt)
            for tt in range(NSUB):
                fg = S["fgs"][tt]
                out_t = opool.tile([P, H], BF16, name=f"o{si}_{tt}",
                                   tag="outt")
                if fast_ln is not None:
                    # u1 = (x - mu1) * igA computed as x*igA + (-mu1*igA)
                    # on ACT (per-partition scale/bias); same for fg; then
                    # one DVE add. igA/igB already carry the gains.
                    bias2 = spool.tile([P, 2], F32, name=f"bs{si}_{tt}",
                                       tag="bias2")
                    nc.vector.scalar_tensor_tensor(
                        bias2[:, 0:1], agg[:, tt, 0:1], -1.0,
                        ig[:, 2 * tt:2 * tt + 1], ALU.mult, ALU.mult)
                    nc.vector.scalar_tensor_tensor(
                        bias2[:, 1:2], agg[:, tt, 2:3], -1.0,
                        ig[:, 2 * tt + 1:2 * tt + 2], ALU.mult, ALU.mult)
                    u1 = tpool.tile([P, H], BF16, tag="tmp1")
                    u2 = tpool.tile([P, H], BF16, tag="tmp2")
                    nc.scalar.activation(u1, S["x"][:, tt, :], AF.Identity,
                                         bias=bias2[:, 0:1],
                                         scale=ig[:, 2 * tt:2 * tt + 1])
                    nc.scalar.activation(u2, fg, AF.Identity,
                                         bias=bias2[:, 1:2],
                                         scale=ig[:, 2 * tt + 1:2 * tt + 2])
                    nc.vector.tensor_tensor(out_t, u1, u2, ALU.add)
                else:
                    tmp1 = tpool.tile([P, H], BF16, tag="tmp1")
                    tmp2 = tpool.tile([P, H], BF16, tag="tmp2")
                    # tmp2 = (fg - mu2) * G2;  G2 = w*ln2_g broadcast
                    nc.vector.scalar_tensor_tensor(
                        tmp2, fg, agg[:, tt, 2:3], g2b_sb,
                        ALU.subtract, ALU.mult)
                    # tmp1 = (x - mu1) * G1;  G1 = (1-w)*ln1_g
                    nc.vector.scalar_tensor_tensor(
                        tmp1, S["x"][:, tt, :], agg[:, tt, 0:1], g1b_sb,
                        ALU.subtract, ALU.mult)
                    # tmp1 = tmp1 * ig1 + Bc;  Bc = w*ln2_b + (1-w)*ln1_b
                    nc.vector.scalar_tensor_tensor(
                        tmp1, tmp1, ig[:, 2 * tt:2 * tt + 1], bcb_sb,
                        ALU.mult, ALU.add)
                    nc.vector.scalar_tensor_tensor(
                        out_t, tmp2, ig[:, 2 * tt + 1:2 * tt + 2], tmp1,
                        ALU.mult, ALU.add)
                rows = P if (tt == 0 or ntok == 2 * P) else ntok - P
                nc.sync.dma_start(
                    out_d[t0 + tt * P:t0 + tt * P + rows, :],
                    out_t[0:rows, :])

        for si in range(len(SUPS)):
            loads_a(si)
            loads_b(si)
            if si > 0:
                tail_dve(si - 1)
            tail_pe(si)
        tail_dve(len(SUPS) - 1)


_NC_CACHE = {}


def _get_nc(fast_ln=None):
    key = ("fast", fast_ln) if fast_ln is None else (
        "fast", (round(float(fast_ln[0]), 6), round(float(fast_ln[1]), 6)))
    if key not in _NC_CACHE:
        _NC_CACHE[key] = _build_program(fast_ln=fast_ln)
    return _NC_CACHE[key]


def _host_prep(codes1, scales1, b1, codes2, scales2, b2,
               ln1_g, ln1_b, ln2_g, ln2_b, alpha):
    # W1 with 1/13 folded into the f1/f2 column blocks (mean -> sum)
    w1 = _dequant_nf4(codes1, scales1)
    # match reference rounding: dequant result is cast to bf16 first
    w1 = w1.astype(NP_BF16).astype(np.float32)
    w1[:, H:] *= np.float32(1.0 / 13.0)
    w1t = np.ascontiguousarray(w1.T).astype(NP_BF16)

    w2 = _dequant_nf4(codes2, scales2).astype(NP_BF16)
    w2t = np.ascontiguousarray(w2.astype(np.float32).T).astype(NP_BF16)

    b1s = np.ascontiguousarray(
        b1.astype(np.float32).reshape(MO, P).T)  # [P, MO]

    b2b = np.ascontiguousarray(b2.astype(NP_BF16).reshape(1, H))

    a32 = alpha.astype(np.float32)
    w_gate = (1.0 / (1.0 + np.exp(-a32[0]))).astype(NP_BF16)
    one_minus = (NP_BF16(1.0) - w_gate)
    g1 = (one_minus.astype(np.float32) * ln1_g.astype(np.float32))
    g2 = (w_gate.astype(np.float32) * ln2_g.astype(np.float32))
    bc = (w_gate.astype(np.float32) * ln2_b.astype(np.float32)
          + one_minus.astype(np.float32) * ln1_b.astype(np.float32))
    g1b = np.ascontiguousarray(np.broadcast_to(g1.astype(NP_BF16), (P, H)))
    g2b = np.ascontiguousarray(np.broadcast_to(g2.astype(NP_BF16), (P, H)))
    bcb = np.ascontiguousarray(np.broadcast_to(bc.astype(NP_BF16), (P, H)))

    # fast-LN eligibility: uniform positive folded gains, zero folded bias
    fast_ln = None
    g1f = g1.astype(NP_BF16).astype(np.float32)
    g2f = g2.astype(NP_BF16).astype(np.float32)
    if (np.all(g1f == g1f[0]) and np.all(g2f == g2f[0])
            and g1f[0] > 0 and g2f[0] > 0
            and np.all(bc.astype(NP_BF16) == 0)):
        fast_ln = (float(g1f[0]), float(g2f[0]))
    return w1t, w2t, b1s, b2b, g1b, g2b, bcb, fast_ln


def make_in_maps(image_features, hidden, codes1, scales1, b1, codes2, scales2,
                 b2, ln1_g, ln1_b, ln2_g, ln2_b, alpha):
    w1t, w2t, b1s, b2b, g1b, g2b, bcb, fast_ln = _host_prep(
        codes1, scales1, b1, codes2, scales2, b2,
        ln1_g, ln1_b, ln2_g, ln2_b, alpha)
    B = image_features.shape[0]
    in_maps = []
    for c in range(B):
        in_maps.append({
            "x": np.ascontiguousarray(image_features[c]).astype(NP_BF16, copy=False),
            "hid": np.ascontiguousarray(hidden[:, c]).astype(NP_BF16, copy=False),
            "w1t": w1t, "w2t": w2t, "b1s": b1s, "b2b": b2b,
            "g1b": g1b, "g2b": g2b, "bcb": bcb,
        })
    return in_maps, fast_ln


def kernel(image_features, hidden, codes1, scales1, b1, codes2, scales2, b2,
           ln1_g, ln1_b, ln2_g, ln2_b, alpha, _trace=False):
    B, N, Hin = image_features.shape
    assert (B, N, Hin) == (8, NT, H), (B, N, Hin)
    in_maps, fast_ln = make_in_maps(
        image_features, hidden, codes1, scales1, b1, codes2, scales2, b2,
        ln1_g, ln1_b, ln2_g, ln2_b, alpha)
    nc = _get_nc(fast_ln)
    res = bass_utils.run_bass_kernel_spmd(
        nc, in_maps, core_ids=list(range(8)), trace=_trace)
    out = np.stack([res.results[c]["out"] for c in range(8)])
    if _trace:
        kernel._last_results = res
    return out.astype(image_features.dtype, copy=False)


# revision 57
# speedup vs baseline: 1.7716x; 1.0126x over previous
"""Trainium2 Bass kernel for nn_Connector_77738908057780 (dense_mlp).

Computation (see reference):
  x   = image_features                      [B, N, H]    bf16
  f1  = mean(hidden[0:13],  axis=0)         [B, N, H]
  f2  = mean(hidden[13:26], axis=0)         [B, N, H]
  cat = concat([x, f1, f2], -1)             [B, N, 3H]
  h   = gelu(cat @ W1.T + b1)               W1 = nf4_dequant(codes1, scales1) [H, 3H]
  fg  = h @ W2.T + b2                       W2 = nf4_dequant(codes2, scales2) [H, H]
  out = w * LN(fg) + (1-w) * LN(x),         w = sigmoid(alpha)

Sharding: data-parallel over batch B=8 -> one batch element per NeuronCore.

Per-core schedule (v5, software-pipelined over 3 token supertiles of 256):
  - sync HWDGE queue carries only the dependency-free streaming loads in
    time order (x tile, 26 hidden layers per supertile, with the later
    GEMM1/GEMM2 weight chunks spliced in at the position they are first
    needed); the scalar HWDGE queue carries the first weight chunk +
    consts up front, then output stores.
  - 26-layer sums on DVE add-chains tracking the DMA stream; cat^T built
    by PE identity-transposes (PSUM) + ACT copies, so the DMA queues see
    no xbar transposes (dma_start_transpose drains its queue as deadlock
    protection, head-blocking the stream).
  - GEMM1 is k-OUTER with 9 m-chunk PSUM accumulators packed 2-per-bank
    (5 banks): the k=0..8 (x) matmuls run as soon as x lands, k=9..17 on
    s1, k=18..26 on s2 — so only ~10us of GEMM1 trails each supertile's
    stream instead of the whole GEMM.
  - GELU(+b1 per-partition bias) on ACT -> g^T feeds GEMM2 as stationary,
    producing fg token-major.
  - LN stats via DVE bn_stats/bn_aggr; rsqrt via DVE reciprocal + ACT
    sqrt. When the folded LN gains are uniform (ln*_g constant, gate
    scalar — true for this model's inputs) the normalize+gate combine
    runs as two ACT ops (per-partition scale/bias) + one DVE add; the
    general path (4 DVE scalar_tensor_tensor ops) is kept as a fallback
    program variant.

The last supertile overlaps the previous by 39 tokens (computed twice,
stored twice, identical) so every tile is a full 128-partition tile.

NF4 dequant of the (small, replicated) weights is host-side weight prep;
the bf16 weights are less DMA traffic than the int32 codes.
"""

import os
import sys

import numpy as np
import ml_dtypes

for _p in ("/opt/trn_rl_repo", "/root/.axon_site/_ro/trn_rl_repo"):
    if os.path.isdir(_p) and _p not in sys.path:
        sys.path.insert(0, _p)

import concourse.bass as bass
import concourse.mybir as mybir
import concourse.tile as tile
from concourse import bacc
from concourse import bass_utils
from concourse import masks

BF16 = mybir.dt.bfloat16
F32 = mybir.dt.float32
AF = mybir.ActivationFunctionType
ALU = mybir.AluOpType

NP_BF16 = ml_dtypes.bfloat16

P = 128
H = 1152
H3 = 3456
NT = 729          # tokens per core (N); B=8 cores
L = 26
KO1 = H3 // P     # 27 k-tiles for GEMM1
KO2 = H // P      # 9 k-tiles for GEMM2
MO = H // P       # 9 output-feature tiles
EPS = 1e-5
NCHUNK = 3        # fg free-dim chunks of 384
CH = H // NCHUNK  # 384

# Supertiles of exactly 256 tokens; the last overlaps the previous by 39
# tokens (473..511 computed twice, identical values stored twice) so that
# every DMA/compute tile is a full 128-partition tile (729 is not a
# multiple of 128; partial tiles would leave stale SBUF rows flowing into
# the transposes and stats ops).
SUPS = [(0, 256), (256, 256), (473, 256)]
NSUB = 2

NF4_CODEBOOK = np.array([
    -1.0, -0.6961928009986877, -0.5250730514526367, -0.39491748809814453,
    -0.28444138169288635, -0.18477343022823334, -0.09105003625154495, 0.0,
    0.07958029955625534, 0.16093020141124725, 0.24611230194568634,
    0.33791524171829224, 0.4407098591327667, 0.5626170039176941,
    0.7229568362236023, 1.0], dtype=np.float32)

BLOCK = 64


def _dequant_nf4(codes, scales):
    """Match reference: codebook lookup * per-64-block absmax, cast bf16."""
    out_f, in_f = codes.shape
    w = NF4_CODEBOOK[codes].reshape(out_f, in_f // BLOCK, BLOCK)
    w = w * scales[:, :, None].astype(np.float32)
    return w.reshape(out_f, in_f)  # float32 (caller casts)


def _build_program(act=AF.Gelu, fast_ln=None):
    """fast_ln: None for the general LN-combine path, or (g1v, g2v) scalar
    gains (>0, with zero combined bias) for the ACT-based fast path."""
    nc = bacc.Bacc(
        "TRN2",
        target_bir_lowering=False,
        debug=False,
        num_devices=1,
    )
    x_d = nc.dram_tensor("x", (NT, H), BF16, kind="ExternalInput").ap()
    hid_d = nc.dram_tensor("hid", (L, NT, H), BF16, kind="ExternalInput").ap()
    w1t_d = nc.dram_tensor("w1t", (H3, H), BF16, kind="ExternalInput").ap()
    w2t_d = nc.dram_tensor("w2t", (H, H), BF16, kind="ExternalInput").ap()
    b1s_d = nc.dram_tensor("b1s", (P, MO), F32, kind="ExternalInput").ap()
    b2b_d = nc.dram_tensor("b2b", (1, H), BF16, kind="ExternalInput").ap()
    g1b_d = nc.dram_tensor("g1b", (P, H), BF16, kind="ExternalInput").ap()
    g2b_d = nc.dram_tensor("g2b", (P, H), BF16, kind="ExternalInput").ap()
    bcb_d = nc.dram_tensor("bcb", (P, H), BF16, kind="ExternalInput").ap()
    out_d = nc.dram_tensor("out", (NT, H), BF16, kind="ExternalOutput").ap()

    with tile.TileContext(nc) as tc:
        _program(nc, tc, x_d, hid_d, w1t_d, w2t_d, b1s_d, b2b_d,
                 g1b_d, g2b_d, bcb_d, out_d, act, fast_ln)

    nc.compile()
    return nc


def _program(nc, tc, x_d, hid_d, w1t_d, w2t_d, b1s_d, b2b_d, g1b_d, g2b_d,
             bcb_d, out_d, act=AF.Gelu, fast_ln=None):
    with (
        tc.tile_pool(name="consts", bufs=1) as cpool,
        tc.tile_pool(name="hl", bufs=9) as hpool,
        tc.tile_pool(name="acc", bufs=3) as apool,
        tc.tile_pool(name="cat", bufs=2) as catpool,
        tc.tile_pool(name="gt", bufs=2) as gpool,
        tc.tile_pool(name="xn", bufs=2) as xpool,
        tc.tile_pool(name="fg", bufs=4) as fgpool,
        tc.tile_pool(name="outp", bufs=2) as opool,
        tc.tile_pool(name="stats", bufs=3) as spool,
        tc.tile_pool(name="tmp", bufs=2) as tpool,
        tc.tile_pool(name="psA", bufs=5, space="PSUM") as ps1pool,
        tc.tile_pool(name="psB", bufs=2, space="PSUM") as ps2pool,
        tc.tile_pool(name="psT", bufs=1, space="PSUM") as tpspool,
    ):
        # ---- first GEMM1 weight chunk + small consts on the scalar HWDGE
        # queue up front; the remaining weight chunks are spliced into the
        # sync stream (see accum_half inline=).
        w1t_sb = cpool.tile([P, KO1, H], BF16, name="w1t")
        w1t_r = w1t_d.rearrange("(ko p) n -> p ko n", p=P)
        nc.scalar.dma_start(w1t_sb[:, 0:4, :], w1t_r[:, 0:4, :])
        nc.scalar.dma_start(w1t_sb[:, 4:9, :], w1t_r[:, 4:9, :])
        w2t_sb = cpool.tile([P, KO2, H], BF16, name="w2t")
        b1_sb = cpool.tile([P, MO], F32, name="b1s")
        nc.scalar.dma_start(b1_sb, b1s_d)
        b2b_sb = cpool.tile([1, H], BF16, name="b2b")
        nc.scalar.dma_start(b2b_sb, b2b_d[0:1, :])
        if fast_ln is None:
            g1b_sb = cpool.tile([P, H], BF16, name="g1b")
            nc.scalar.dma_start(g1b_sb, g1b_d)
            g2b_sb = cpool.tile([P, H], BF16, name="g2b")
            nc.scalar.dma_start(g2b_sb, g2b_d)
            bcb_sb = cpool.tile([P, H], BF16, name="bcb")
            nc.scalar.dma_start(bcb_sb, bcb_d)
        ident = cpool.tile([P, P], BF16, name="ident")
        masks.make_identity(nc, ident)
        ones_sb = cpool.tile([1, P], BF16, name="ones1")
        nc.vector.memset(ones_sb, 1.0)

        st = [dict() for _ in SUPS]

        def pe_transpose(catT, tt, ko0, src):
            """catT[:, tt, ko0+k, :] = src[:, k*P:(k+1)*P].T for k in 0..MO.

            PE identity-transpose into PSUM, ACT copies back to SBUF. Keeps
            the HWDGE queues free of xbar transposes (dma_start_transpose
            drains its queue as deadlock protection; measured, it stalls
            the streaming loads badly on either HWDGE queue).
            """
            for g in range(3):
                tps = tpspool.tile([P, 3, P], BF16, tag="tps")
                for c in range(3):
                    k = g * 3 + c
                    nc.tensor.transpose(tps[:, c, :],
                                        src[:, k * P:(k + 1) * P], ident)
                nc.scalar.copy(catT[:, tt, ko0 + 3 * g:ko0 + 3 * (g + 1), :],
                               tps)

        def gemm1_ks(si, k0, k1):
            """GEMM1 matmuls for k-chunks [k0, k1) across all 9 m-chunks."""
            S = st[si]
            catT = S["catT"]
            ps1s = S["ps1s"]
            # Two m-chunks share each PSUM bank, so the hardware's
            # bank-granular start-zeroing cannot be used: the tiles are
            # memset once (loads_a) and every matmul pure-accumulates.
            for kk in range(k0, k1):
                for mm in range(MO):
                    dst = ps1s[mm // 2][:, mm % 2, :]
                    nc.tensor.matmul(
                        dst.rearrange("p (a b) -> p a b", a=NSUB),
                        lhsT=w1t_sb[:, kk, mm * P:(mm + 1) * P],
                        rhs=catT[:, :, kk, :],
                        start=False,
                        stop=(kk == KO1 - 1),
                        skip_group_check=True,
                    )

        def load_token_tile(dst, src2d, t0, ntok):
            """Load [ntok, H] DRAM rows into dst [P, 2, H] token-major.
            For partial supertiles the caller pre-memsets rows ntok-P..P of
            subtile 1 (gpsimd), so every downstream 128-partition op reads
            initialized data."""
            if ntok == 2 * P:
                nc.sync.dma_start(
                    dst, src2d[t0:t0 + 2 * P, :].rearrange(
                        "(s p) f -> p s f", p=P))
            else:
                nc.sync.dma_start(dst[:, 0, :], src2d[t0:t0 + P, :])
                nc.sync.dma_start(dst[0:ntok - P, 1, :],
                                  src2d[t0 + P:t0 + ntok, :])

        def accum_half(si, half, insert_s1T=False, inline=None):
            """Load 13 layers, DVE-chain them into an acc tile.

            ``inline`` maps layer-offset -> callable emitted right after that
            layer's load (used to splice weight-chunk DMAs into the sync
            stream at the position they are first needed).
            """
            t0, ntok = SUPS[si]
            l0 = 13 * half
            acc = apool.tile([P, NSUB, H], BF16, name=f"s{si}_{half}",
                             tag="acc")
            def layer_tile(i):
                lt = hpool.tile([P, NSUB, H], BF16, name=f"hl{si}_{l0+i}",
                                tag="hl")
                load_token_tile(lt, hid_d[l0 + i], t0, ntok)
                if inline and i in inline:
                    inline[i]()
                return lt

            lts = [layer_tile(0), layer_tile(1)]
            nc.vector.tensor_add(acc, lts[0], lts[1])
            for i in range(2, 13):
                lt = layer_tile(i)
                nc.vector.tensor_add(acc, acc, lt)
                if insert_s1T and i == 3:
                    # s1 is long done by now; transpose it mid-stream and
                    # run GEMM1's k=9..17 before s2 lands.
                    S = st[si]
                    for tt in range(NSUB):
                        pe_transpose(S["catT"], tt, MO, S["s1"][:, tt, :])
                    gemm1_ks(si, MO, 2 * MO)
            return acc

        def loads_a(si):
            t0, ntok = SUPS[si]
            S = st[si]
            catT = catpool.tile([P, NSUB, KO1, P], BF16, name=f"cat{si}",
                                tag="catT")
            S["catT"] = catT
            S["ps1s"] = [ps1pool.tile([P, 2, NSUB * P], F32, tag="ps1",
                                      name=f"ps1_{si}_{j}")
                         for j in range(5)]
            for t in S["ps1s"]:
                nc.vector.memset(t, 0.0)
            x_nat = xpool.tile([P, NSUB, H], BF16, name=f"x{si}", tag="xnat")
            S["x"] = x_nat
            load_token_tile(x_nat, x_d, t0, ntok)
            for tt in range(NSUB):
                pe_transpose(catT, tt, 0, x_nat[:, tt, :])
            gemm1_ks(si, 0, MO)
            agg = spool.tile([P, NSUB, 4], F32, name=f"agg{si}", tag="agg")
            S["agg"] = agg
            for tt in range(NSUB):
                bnx = spool.tile([P, 3, 6], F32, name=f"bnx{si}_{tt}",
                                 tag="bnx")
                for c in range(NCHUNK):
                    nc.vector.bn_stats(bnx[:, c, :],
                                       x_nat[:, tt, c * CH:(c + 1) * CH])
                nc.vector.bn_aggr(agg[:, tt, 0:2], bnx)
            inline = None
            if si == 0:
                inline = {6: lambda: nc.sync.dma_start(
                    w1t_sb[:, 9:18, :], w1t_r[:, 9:18, :])}
            S["s1"] = accum_half(si, 0, inline=inline)

        def loads_b(si):
            S = st[si]
            inline = None
            if si == 0:
                inline = {
                    2: lambda: nc.sync.dma_start(
                        w1t_sb[:, 18:27, :], w1t_r[:, 18:27, :]),
                    7: lambda: nc.sync.dma_start(
                        w2t_sb, w2t_d.rearrange("(ko p) n -> p ko n", p=P)),
                }
            S["s2"] = accum_half(si, 1, insert_s1T=True, inline=inline)
            for tt in range(NSUB):
                pe_transpose(S["catT"], tt, 2 * MO, S["s2"][:, tt, :])

        def tail_pe(si):
            S = st[si]
            gemm1_ks(si, 2 * MO, 3 * MO)
            gT = gpool.tile([P, MO, NSUB * P], BF16, name=f"gT{si}", tag="gT")
            S["gT"] = gT
            for mm in range(MO):
                nc.scalar.activation(gT[:, mm, :],
                                     S["ps1s"][mm // 2][:, mm % 2, :],
                                     act, bias=b1_sb[:, mm:mm + 1])
            fgs = []
            for tt in range(NSUB):
                fg = fgpool.tile([P, H], BF16, name=f"fg{si}_{tt}", tag="fg")
                fgs.append(fg)
                for nn in range(NCHUNK):
                    ps2 = ps2pool.tile([P, CH], F32, tag="ps2")
                    for kk in range(KO2):
                        nc.tensor.matmul(
                            ps2,
                            lhsT=gT[:, kk, tt * P:(tt + 1) * P],
                            rhs=w2t_sb[:, kk, nn * CH:(nn + 1) * CH],
                            start=(kk == 0),
                            stop=False,
                        )
                    # b2 bias as a k=1 ones-row matmul; evacuation on ACT
                    # (keeps the bias-add off the saturated DVE)
                    nc.tensor.matmul(
                        ps2,
                        lhsT=ones_sb,
                        rhs=b2b_sb[:, nn * CH:(nn + 1) * CH],
                        start=False,
                        stop=True,
                    )
                    nc.scalar.copy(fg[:, nn * CH:(nn + 1) * CH], ps2)
            S["fgs"] = fgs

        def tail_dve(si):
            t0, ntok = SUPS[si]
            S = st[si]
            agg = S["agg"]
            rpack = spool.tile([P, 2 * NSUB], F32, name=f"rp{si}", tag="rpack")
            for tt in range(NSUB):
                fg = S["fgs"][tt]
                bnf = spool.tile([P, 3, 6], F32, name=f"bnf{si}_{tt}",
                                 tag="bnf")
                for c in range(NCHUNK):
                    nc.vector.bn_stats(bnf[:, c, :],
                                       fg[:, c * CH:(c + 1) * CH])
                nc.vector.bn_aggr(agg[:, tt, 2:4], bnf)
                if fast_ln is None:
                    nc.vector.tensor_scalar_add(rpack[:, 2 * tt:2 * tt + 1],
                                                agg[:, tt, 1:2], EPS)
                    nc.vector.tensor_scalar_add(
                        rpack[:, 2 * tt + 1:2 * tt + 2],
                        agg[:, tt, 3:4], EPS)
                else:
                    g1v, g2v = fast_ln
                    # rsqrt((var+eps)/g^2) = g * rsqrt(var+eps)
                    nc.vector.tensor_scalar(
                        rpack[:, 2 * tt:2 * tt + 1], agg[:, tt, 1:2],
                        EPS, float(1.0 / (g1v * g1v)), ALU.add, ALU.mult)
                    nc.vector.tensor_scalar(
                        rpack[:, 2 * tt + 1:2 * tt + 2], agg[:, tt, 3:4],
                        EPS, float(1.0 / (g2v * g2v)), ALU.add, ALU.mult)
            ig = spool.tile([P, 2 * NSUB], F32, name=f"ig{si}", tag="ig")
            nc.vector.reciprocal(ig, rpack)
            nc.scalar.activation(ig, ig, AF.Sqrt)
            for tt in range(NSUB):
                fg = S["fgs"][tt]
                out_t = opool.tile([P, H], BF16, name=f"o{si}_{tt}",
                                   tag="outt")
                if fast_ln is not None:
                    # u1 = (x - mu1) * igA computed as x*igA + (-mu1*igA)
                    # on ACT (per-partition scale/bias); same for fg; then
                    # one DVE add. igA/igB already carry the gains.
                    bias2 = spool.tile([P, 2], F32, name=f"bs{si}_{tt}",
                                       tag="bias2")
                    nc.vector.scalar_tensor_tensor(
                        bias2[:, 0:1], agg[:, tt, 0:1], -1.0,
                        ig[:, 2 * tt:2 * tt + 1], ALU.mult, ALU.mult)
                    nc.vector.scalar_tensor_tensor(
                        bias2[:, 1:2], agg[:, tt, 2:3], -1.0,
                        ig[:, 2 * tt + 1:2 * tt + 2], ALU.mult, ALU.mult)
                    u1 = tpool.tile([P, H], BF16, tag="tmp1")
                    u2 = tpool.tile([P, H], BF16, tag="tmp2")
                    nc.scalar.activation(u1, S["x"][:, tt, :], AF.Identity,
                                         bias=bias2[:, 0:1],
                                         scale=ig[:, 2 * tt:2 * tt + 1])
                    nc.scalar.activation(u2, fg, AF.Identity,
                                         bias=bias2[:, 1:2],
                                         scale=ig[:, 2 * tt + 1:2 * tt + 2])
                    nc.vector.tensor_tensor(out_t, u1, u2, ALU.add)
                else:
                    tmp1 = tpool.tile([P, H], BF16, tag="tmp1")
                    tmp2 = tpool.tile([P, H], BF16, tag="tmp2")
                    # tmp2 = (fg - mu2) * G2;  G2 = w*ln2_g broadcast
                    nc.vector.scalar_tensor_tensor(
                        tmp2, fg, agg[:, tt, 2:3], g2b_sb,
                        ALU.subtract, ALU.mult)
                    # tmp1 = (x - mu1) * G1;  G1 = (1-w)*ln1_g
                    nc.vector.scalar_tensor_tensor(
                        tmp1, S["x"][:, tt, :], agg[:, tt, 0:1], g1b_sb,
                        ALU.subtract, ALU.mult)
                    # tmp1 = tmp1 * ig1 + Bc;  Bc = w*ln2_b + (1-w)*ln1_b
                    nc.vector.scalar_tensor_tensor(
                        tmp1, tmp1, ig[:, 2 * tt:2 * tt + 1], bcb_sb,
                        ALU.mult, ALU.add)
                    nc.vector.scalar_tensor_tensor(
                        out_t, tmp2, ig[:, 2 * tt + 1:2 * tt + 2], tmp1,
                        ALU.mult, ALU.add)
                rows = P if (tt == 0 or ntok == 2 * P) else ntok - P
                nc.sync.dma_start(
                    out_d[t0 + tt * P:t0 + tt * P + rows, :],
                    out_t[0:rows, :])

        for si in range(len(SUPS)):
            loads_a(si)
            loads_b(si)
            if si > 0:
                tail_dve(si - 1)
            tail_pe(si)
        tail_dve(len(SUPS) - 1)


_NC_CACHE = {}


def _get_nc(fast_ln=None):
    key = ("fast", fast_ln) if fast_ln is None else (
        "fast", (round(float(fast_ln[0]), 6), round(float(fast_ln[1]), 6)))
    if key not in _NC_CACHE:
        _NC_CACHE[key] = _build_program(fast_ln=fast_ln)
    return _NC_CACHE[key]


def _host_prep(codes1, scales1, b1, codes2, scales2, b2,
               ln1_g, ln1_b, ln2_g, ln2_b, alpha):
    # W1 with 1/13 folded into the f1/f2 column blocks (mean -> sum)
    w1 = _dequant_nf4(codes1, scales1)
    # match reference rounding: dequant result is cast to bf16 first
    w1 = w1.astype(NP_BF16).astype(np.float32)
    w1[:, H:] *= np.float32(1.0 / 13.0)
    w1t = np.ascontiguousarray(w1.T).astype(NP_BF16)

    w2 = _dequant_nf4(codes2, scales2).astype(NP_BF16)
    w2t = np.ascontiguousarray(w2.astype(np.float32).T).astype(NP_BF16)

    b1s = np.ascontiguousarray(
        b1.astype(np.float32).reshape(MO, P).T)  # [P, MO]

    b2b = np.ascontiguousarray(b2.astype(NP_BF16).reshape(1, H))

    a32 = alpha.astype(np.float32)
    w_gate = (1.0 / (1.0 + np.exp(-a32[0]))).astype(NP_BF16)
    one_minus = (NP_BF16(1.0) - w_gate)
    g1 = (one_minus.astype(np.float32) * ln1_g.astype(np.float32))
    g2 = (w_gate.astype(np.float32) * ln2_g.astype(np.float32))
    bc = (w_gate.astype(np.float32) * ln2_b.astype(np.float32)
          + one_minus.astype(np.float32) * ln1_b.astype(np.float32))
    g1b = np.ascontiguousarray(np.broadcast_to(g1.astype(NP_BF16), (P, H)))
    g2b = np.ascontiguousarray(np.broadcast_to(g2.astype(NP_BF16), (P, H)))
    bcb = np.ascontiguousarray(np.broadcast_to(bc.astype(NP_BF16), (P, H)))

    # fast-LN eligibility: uniform positive folded gains, zero folded bias
    fast_ln = None
    g1f = g1.astype(NP_BF16).astype(np.float32)
    g2f = g2.astype(NP_BF16).astype(np.float32)
    if (np.all(g1f == g1f[0]) and np.all(g2f == g2f[0])
            and g1f[0] > 0 and g2f[0] > 0
            and np.all(bc.astype(NP_BF16) == 0)):
        fast_ln = (float(g1f[0]), float(g2f[0]))
    return w1t, w2t, b1s, b2b, g1b, g2b, bcb, fast_ln


def make_in_maps(image_features, hidden, codes1, scales1, b1, codes2, scales2,
                 b2, ln1_g, ln1_b, ln2_g, ln2_b, alpha):
    w1t, w2t, b1s, b2b, g1b, g2b, bcb, fast_ln = _host_prep(
        codes1, scales1, b1, codes2, scales2, b2,
        ln1_g, ln1_b, ln2_g, ln2_b, alpha)
    B = image_features.shape[0]
    in_maps = []
    for c in range(B):
        in_maps.append({
            "x": np.ascontiguousarray(image_features[c]).astype(NP_BF16, copy=False),
            "hid": np.ascontiguousarray(hidden[:, c]).astype(NP_BF16, copy=False),
            "w1t": w1t, "w2t": w2t, "b1s": b1s, "b2b": b2b,
            "g1b": g1b, "g2b": g2b, "bcb": bcb,
        })
    return in_maps, fast_ln


def kernel(image_features, hidden, codes1, scales1, b1, codes2, scales2, b2,
           ln1_g, ln1_b, ln2_g, ln2_b, alpha, _trace=False):
    B, N, Hin = image_features.shape
    assert (B, N, Hin) == (8, NT, H), (B, N, Hin)
    in_maps, fast_ln = make_in_maps(
        image_features, hidden, codes1, scales1, b1, codes2, scales2, b2,
        ln1_g, ln1_b, ln2_g, ln2_b, alpha)
    nc = _get_nc(fast_ln)
    res = bass_utils.run_bass_kernel_spmd(
        nc, in_maps, core_ids=list(range(8)), trace=_trace)
    out = np.stack([res.results[c]["out"] for c in range(8)])
    if _trace:
        kernel._last_results = res
    return out.astype(image_features.dtype, copy=False)
